# revision 1
# baseline (speedup 1.0000x reference)
"""DiffusionNet forward on 8 Trainium2 NeuronCores.

Strategy
--------
B=4 samples, 2 cores per sample, each core owns half the mesh nodes
(20000, zero-padded to 20480).  All cross-node coupling flows through the
K=128 spectral bottleneck:

  * SpMM is eliminated on-device: gX = G @ x_diffuse = (G @ evecs) @ S with
    S = coefs * x_spec, so host precomputes GXe = G @ evecs once per sample
    (exact associativity; measured 4e-7 rel err).
  * Per block: partial x_spec^T = sum_n x[n,:]^T evm[n,:] over owned nodes
    (PE accumulation), pairwise AllReduce of the [C,K] partial (64KB), then a
    fused channel-major sweep over node chunks computes x_diffuse, gX, gY,
    Breal, Bimag, grad_feat, the MLP and the residual without touching HBM
    for intermediates.

Layouts: per-node tensors live channel-major ([C, n]) in SBUF; x carried in
fp32 (+ a bf16 shadow for matmul operands), streamed operands in bf16.
"""

import sys
import numpy as np
import ml_dtypes

for _p in ("/opt/trn_rl_repo", "/root/.axon_site/_ro/trn_rl_repo"):
    if _p not in sys.path:
        sys.path.append(_p)

import concourse.bass as bass
import concourse.bacc as bacc
import concourse.tile as tile
import concourse.mybir as mybir
from concourse.bass_utils import run_bass_kernel_spmd
from concourse.masks import make_identity

BF = mybir.dt.bfloat16
F32 = mybir.dt.float32
F32R = mybir.dt.float32r
AF = mybir.ActivationFunctionType
ALU = mybir.AluOpType

B, N, E, K = 4, 40000, 240000, 128
C = 128
NB = 4          # diffusion blocks
NCORES = 8
NH = N // 2     # nodes per core (half sample)
CH = 512        # node chunk (matmul free dim)
NHP = 20480     # padded nodes per core: 40 chunks * 512 = 160 tiles * 128
NCH = NHP // CH
NT = NHP // 128
PAIRS = [[0, 1], [2, 3], [4, 5], [6, 7]]

bf16 = ml_dtypes.bfloat16


# ----------------------------------------------------------------- host side

def _spmm_mat(rows, cols, vals, M):
    """(COO [N,N] with given pattern) @ M, dense M [N,k]. Pure numpy."""
    out = np.zeros((N, M.shape[1]), np.float32)
    perm = np.argsort(rows, kind="stable")
    contrib = (vals[:, None] * M[cols]).astype(np.float32)[perm]
    rs = rows[perm]
    uniq, starts = np.unique(rs, return_index=True)
    out[uniq] = np.add.reduceat(contrib, starts, axis=0)
    return out


def host_prep(inputs, nhp=NHP, nb=NB):
    """Build the 8 per-core input dicts."""
    x_in = np.asarray(inputs["x_in"], np.float32)
    mass = np.asarray(inputs["mass"], np.float32)
    evals = np.asarray(inputs["evals"], np.float32)
    evecs = np.asarray(inputs["evecs"], np.float32)
    rows = np.asarray(inputs["rows"])
    cols = np.asarray(inputs["cols"])
    gX_vals = np.asarray(inputs["gradX_vals"], np.float32)
    gY_vals = np.asarray(inputs["gradY_vals"], np.float32)
    w_first = np.asarray(inputs["w_first"], np.float32)
    b_first = np.asarray(inputs["b_first"], np.float32)
    diff_time = np.asarray(inputs["diff_time"], np.float32)
    A_re = np.asarray(inputs["A_re"], np.float32)
    A_im = np.asarray(inputs["A_im"], np.float32)
    mlp_w0 = np.asarray(inputs["mlp_w0"], np.float32)
    w1 = np.asarray(inputs["mlp_w1"], np.float32)
    w2 = np.asarray(inputs["mlp_w2"], np.float32)
    b0 = np.asarray(inputs["mlp_b0"], np.float32)
    b1 = np.asarray(inputs["mlp_b1"], np.float32)
    b2 = np.asarray(inputs["mlp_b2"], np.float32)
    w_last = np.asarray(inputs["w_last"], np.float32)
    b_last = np.asarray(inputs["b_last"], np.float32)

    nh = NH

    shared = dict(
        Are=A_re[:nb],
        Aim=A_im[:nb],
        w0af=np.ascontiguousarray(mlp_w0[:nb, 0:C]),
        w0bf=np.ascontiguousarray(mlp_w0[:nb, C:2 * C]),
        w0c=mlp_w0[:nb, 2 * C:3 * C].astype(bf16),
        w1=w1[:nb].astype(bf16),
        w2=w2[:nb].astype(bf16),
        b0=b0[:nb].reshape(nb, C, 1),
        b1=b1[:nb].reshape(nb, C, 1),
        b2=b2[:nb].reshape(nb, C, 1),
        wlastf=w_last,
        blast=b_last.reshape(3, 1),
    )

    in_maps = []
    for b in range(B):
        ev = evecs[b]
        evm_full = ev * mass[b][:, None]
        GXe = _spmm_mat(rows, cols, gX_vals[b], ev)
        GYe = _spmm_mat(rows, cols, gY_vals[b], ev)
        x0_full = x_in[b] @ w_first + b_first
        # coefsT[i][c,k] = exp(-evals[k] * diff_time[i][c])
        coefsT = np.exp(-evals[b][None, None, :]
                        * diff_time[:nb, :, None]).astype(np.float32)
        for h in range(2):
            sl = slice(h * nh, (h + 1) * nh)

            def padT(M):  # [nh, K] -> [K, nhp]
                out = np.zeros((M.shape[1], nhp), np.float32)
                out[:, :nh] = M[sl].T
                return out

            evmP = np.zeros((nhp, K), np.float32)
            evmP[:nh] = evm_full[sl]
            evm4 = evmP.reshape(nhp // 512, 4, 128, K).transpose(0, 2, 1, 3) \
                       .reshape(nhp // 512, 128, 512)
            x0T = padT(x0_full)
            in_maps.append(dict(
                evm4=evm4.astype(bf16),
                evT=padT(ev).astype(bf16),
                gxT=padT(GXe).astype(bf16),
                gyT=padT(GYe).astype(bf16),
                x0T=x0T,
                coefsT=coefsT,
                **shared,
            ))
    return in_maps


# --------------------------------------------------------------- device side

def build_nc(nb=NB, nch=NCH, ncores=NCORES, collective=True):
    nhp = nch * CH
    nt = nhp // 128
    nc = bacc.Bacc("TRN2", target_bir_lowering=False, debug=False,
                   enable_asserts=True, num_devices=ncores)

    evm4 = nc.dram_tensor("evm4", [nch, 128, 512], BF, kind="ExternalInput")
    evT = nc.dram_tensor("evT", [K, nhp], BF, kind="ExternalInput")
    gxT = nc.dram_tensor("gxT", [K, nhp], BF, kind="ExternalInput")
    gyT = nc.dram_tensor("gyT", [K, nhp], BF, kind="ExternalInput")
    x0T = nc.dram_tensor("x0T", [C, nhp], F32, kind="ExternalInput")
    coefsT = nc.dram_tensor("coefsT", [nb, C, K], F32, kind="ExternalInput")
    Are = nc.dram_tensor("Are", [nb, C, C], F32, kind="ExternalInput")
    Aim = nc.dram_tensor("Aim", [nb, C, C], F32, kind="ExternalInput")
    w0af = nc.dram_tensor("w0af", [nb, C, C], F32, kind="ExternalInput")
    w0bf = nc.dram_tensor("w0bf", [nb, C, C], F32, kind="ExternalInput")
    w0c = nc.dram_tensor("w0c", [nb, C, C], BF, kind="ExternalInput")
    w1 = nc.dram_tensor("w1", [nb, C, C], BF, kind="ExternalInput")
    w2 = nc.dram_tensor("w2", [nb, C, C], BF, kind="ExternalInput")
    b0 = nc.dram_tensor("b0", [nb, C, 1], F32, kind="ExternalInput")
    b1 = nc.dram_tensor("b1", [nb, C, 1], F32, kind="ExternalInput")
    b2 = nc.dram_tensor("b2", [nb, C, 1], F32, kind="ExternalInput")
    wlastf = nc.dram_tensor("wlastf", [C, 3], F32, kind="ExternalInput")
    blast = nc.dram_tensor("blast", [3, 1], F32, kind="ExternalInput")
    yT = nc.dram_tensor("yT", [3, nhp], F32, kind="ExternalOutput")

    with tile.TileContext(nc) as tc:
        with (
            tc.tile_pool(name="consts", bufs=1) as consts,
            tc.tile_pool(name="xpool", bufs=1) as xpool,
            tc.tile_pool(name="stream", bufs=4) as stream,
            tc.tile_pool(name="csb", bufs=3) as csb,
            tc.tile_pool(name="smalls", bufs=2) as smalls,
            tc.tile_pool(name="mm_ps", bufs=7, space="PSUM") as mm_ps,
            tc.tile_pool(name="small_ps", bufs=1, space="PSUM") as small_ps,
            tc.tile_pool(name="dram", bufs=2, space="DRAM") as dram,
        ):
            ident_bf = consts.tile([128, 128], BF, tag="identb")
            make_identity(nc, ident_bf[:])
            ident_f = consts.tile([128, 128], F32, tag="identf")
            make_identity(nc, ident_f[:])

            def cload(src, shape, dt, tag):
                t = consts.tile(shape, dt, tag=tag)
                nc.sync.dma_start(t[:], src)
                return t

            Are_s = [cload(Are[i], [C, C], F32, f"Are{i}") for i in range(nb)]
            Aim_s = [cload(Aim[i], [C, C], F32, f"Aim{i}") for i in range(nb)]
            coefsT_s = [cload(coefsT[i], [C, K], F32, f"cf{i}") for i in range(nb)]
            w0af_s = [cload(w0af[i], [C, C], F32, f"w0af{i}") for i in range(nb)]
            w0bf_s = [cload(w0bf[i], [C, C], F32, f"w0bf{i}") for i in range(nb)]
            w0c_s = [cload(w0c[i], [C, C], BF, f"w0c{i}") for i in range(nb)]
            w1_s = [cload(w1[i], [C, C], BF, f"w1{i}") for i in range(nb)]
            w2_s = [cload(w2[i], [C, C], BF, f"w2{i}") for i in range(nb)]
            b0_s = [cload(b0[i], [C, 1], F32, f"b0{i}") for i in range(nb)]
            b1_s = [cload(b1[i], [C, 1], F32, f"b1{i}") for i in range(nb)]
            b2_s = [cload(b2[i], [C, 1], F32, f"b2{i}") for i in range(nb)]
            wlastf_s = cload(wlastf[:], [C, 3], F32, "wlast")
            blast_s = cload(blast[:], [3, 1], F32, "blast")

            # fp32r copies of the weights used in fp32r matmuls against x
            w0a_r = []
            for i in range(nb):
                t = consts.tile([C, C], F32R, tag=f"w0ar{i}")
                nc.vector.tensor_copy(t[:], w0af_s[i][:])
                w0a_r.append(t)
            wlast_r = consts.tile([C, 3], F32R, tag="wlastr")
            nc.vector.tensor_copy(wlast_r[:], wlastf_s[:])

            xs = []
            for cI in range(nch):
                sl = bass.ts(cI, CH)
                xtmp = stream.tile([C, CH], F32, tag="x0tmp")
                nc.sync.dma_start(xtmp[:], x0T[:, sl])
                xt = xpool.tile([C, CH], F32R, tag=f"x{cI}")
                nc.vector.tensor_copy(xt[:], xtmp[:])
                xs.append(xt)

            for i in range(nb):
                # ---- forward spectral transform: x_spec^T = sum x^T evm ----
                xspec_ps = small_ps.tile([C, K], F32, tag="sps")
                ebuf = None
                for t in range(nt):
                    cI, f = divmod(t, 4)
                    if f == 0:
                        ebuf = stream.tile([128, 512], BF, tag="evm")
                        nc.sync.dma_start(ebuf[:], evm4[cI])
                    tp = mm_ps.tile([128, 128], F32, tag="mm")
                    nc.tensor.transpose(
                        tp[:], xs[cI][:, f * 128:(f + 1) * 128].bitcast(F32),
                        ident_f[:])
                    xt = csb.tile([128, 128], BF, tag="xt")
                    nc.vector.tensor_copy(xt[:], tp[:])
                    nc.tensor.matmul(xspec_ps[:], xt[:],
                                     ebuf[:, f * 128:(f + 1) * 128],
                                     start=(t == 0), stop=(t == nt - 1))

                # coefs multiply commutes with the pairwise sum -> do it
                # before the AllReduce (off the post-collective critical path)
                STf_p = smalls.tile([C, K], F32, tag="xsp")
                nc.vector.tensor_mul(STf_p[:], xspec_ps[:], coefsT_s[i][:])
                if collective:
                    cc_in = dram.tile([C, K], F32, tag="ccin")
                    cc_out = dram.tile([C, K], F32, tag="ccout")
                    nc.sync.dma_start(cc_in[:], STf_p[:])
                    nc.gpsimd.collective_compute(
                        "AllReduce", ALU.add,
                        replica_groups=PAIRS[:ncores // 2],
                        ins=[cc_in.opt()], outs=[cc_out.opt()])
                    STf = smalls.tile([C, K], F32, tag="STf")
                    nc.sync.dma_start(STf[:], cc_out[:])
                else:
                    STf = STf_p

                # ---- S, its A_re/A_im products, S@w0b ----
                S_ps = small_ps.tile([K, C], F32, tag="sps")
                nc.tensor.transpose(S_ps[:], STf[:], ident_f[:])
                S_bf = smalls.tile([K, C], BF, tag="Sbf")
                nc.scalar.activation(S_bf[:], S_ps[:], AF.Copy)
                Sre_ps = small_ps.tile([K, C], F32, tag="sps")
                nc.tensor.matmul(Sre_ps[:], STf[:], Are_s[i][:],
                                 start=True, stop=True)
                Sre_bf = smalls.tile([K, C], BF, tag="Srebf")
                nc.scalar.activation(Sre_bf[:], Sre_ps[:], AF.Copy)
                Sim_ps = small_ps.tile([K, C], F32, tag="sps")
                nc.tensor.matmul(Sim_ps[:], STf[:], Aim_s[i][:],
                                 start=True, stop=True)
                Sim_bf = smalls.tile([K, C], BF, tag="Simbf")
                nc.scalar.activation(Sim_bf[:], Sim_ps[:], AF.Copy)
                nSim_bf = smalls.tile([K, C], BF, tag="nSimbf")
                nc.vector.tensor_scalar_mul(nSim_bf[:], Sim_ps[:], -1.0)
                SW0b_ps = small_ps.tile([K, C], F32, tag="sps")
                nc.tensor.matmul(SW0b_ps[:], STf[:], w0bf_s[i][:],
                                 start=True, stop=True)
                SW0b_bf = smalls.tile([K, C], BF, tag="SW0b")
                nc.scalar.activation(SW0b_bf[:], SW0b_ps[:], AF.Copy)

                # ---- fused per-node sweep ----
                for cI in range(nch):
                    sl = bass.ts(cI, CH)
                    ev_c = stream.tile([K, CH], BF, tag="ev")
                    nc.sync.dma_start(ev_c[:], evT[:, sl])
                    gx_c = stream.tile([K, CH], BF, tag="gx")
                    nc.sync.dma_start(gx_c[:], gxT[:, sl])
                    gy_c = stream.tile([K, CH], BF, tag="gy")
                    nc.sync.dma_start(gy_c[:], gyT[:, sl])

                    gX_ps = mm_ps.tile([C, CH], F32, tag="mm")
                    nc.tensor.matmul(gX_ps[:], S_bf[:], gx_c[:],
                                     start=True, stop=True)
                    gY_ps = mm_ps.tile([C, CH], F32, tag="mm")
                    nc.tensor.matmul(gY_ps[:], S_bf[:], gy_c[:],
                                     start=True, stop=True)
                    Br_ps = mm_ps.tile([C, CH], F32, tag="mm")
                    nc.tensor.matmul(Br_ps[:], Sre_bf[:], gx_c[:],
                                     start=True, stop=False)
                    nc.tensor.matmul(Br_ps[:], nSim_bf[:], gy_c[:],
                                     start=False, stop=True)
                    Bi_ps = mm_ps.tile([C, CH], F32, tag="mm")
                    nc.tensor.matmul(Bi_ps[:], Sre_bf[:], gy_c[:],
                                     start=True, stop=False)
                    nc.tensor.matmul(Bi_ps[:], Sim_bf[:], gx_c[:],
                                     start=False, stop=True)

                    Br_sb = csb.tile([C, CH], BF, tag="Br")
                    nc.scalar.activation(Br_sb[:], Br_ps[:], AF.Copy)
                    Bi_sb = csb.tile([C, CH], BF, tag="Bi")
                    nc.vector.tensor_copy(Bi_sb[:], Bi_ps[:])
                    m1 = csb.tile([C, CH], BF, tag="m1")
                    nc.vector.tensor_mul(m1[:], gX_ps[:], Br_sb[:])
                    m2 = csb.tile([C, CH], BF, tag="m2")
                    nc.vector.tensor_mul(m2[:], gY_ps[:], Bi_sb[:])
                    a1 = csb.tile([C, CH], BF, tag="a1")
                    nc.vector.tensor_add(a1[:], m1[:], m2[:])
                    gf = csb.tile([C, CH], BF, tag="gf")
                    nc.scalar.activation(gf[:], a1[:], AF.Tanh)

                    h0_ps = mm_ps.tile([C, CH], F32, tag="mm")
                    nc.tensor.matmul(h0_ps[:], w0a_r[i][:], xs[cI][:],
                                     start=True, stop=False)
                    nc.tensor.matmul(h0_ps[:], SW0b_bf[:], ev_c[:],
                                     start=False, stop=False)
                    nc.tensor.matmul(h0_ps[:], w0c_s[i][:], gf[:],
                                     start=False, stop=True)
                    h0_sb = csb.tile([C, CH], BF, tag="h0")
                    nc.scalar.activation(h0_sb[:], h0_ps[:], AF.Relu,
                                         bias=b0_s[i][:])
                    h1_ps = mm_ps.tile([C, CH], F32, tag="mm")
                    nc.tensor.matmul(h1_ps[:], w1_s[i][:], h0_sb[:],
                                     start=True, stop=True)
                    h1_sb = csb.tile([C, CH], BF, tag="h1")
                    nc.scalar.activation(h1_sb[:], h1_ps[:], AF.Relu,
                                         bias=b1_s[i][:])
                    h2_ps = mm_ps.tile([C, CH], F32, tag="mm")
                    nc.tensor.matmul(h2_ps[:], w2_s[i][:], h1_sb[:],
                                     start=True, stop=True)
                    # x += h2 + b2 (x carried in fp32r)
                    nc.vector.scalar_tensor_tensor(
                        out=xs[cI][:], in0=h2_ps[:], scalar=b2_s[i][:],
                        in1=xs[cI][:], op0=ALU.add, op1=ALU.add)

            # ---- output head ----
            for cI in range(nch):
                sl = bass.ts(cI, CH)
                y_ps = mm_ps.tile([3, CH], F32, tag="mm")
                nc.tensor.matmul(y_ps[:], wlast_r[:], xs[cI][:],
                                 start=True, stop=True)
                y_sb = csb.tile([3, CH], F32, tag="y")
                nc.vector.tensor_scalar_add(y_sb[:], y_ps[:], blast_s[:])
                nc.sync.dma_start(yT[:, sl], y_sb[:])

    nc.compile()
    return nc


_NC_CACHE = {}


def _get_nc():
    if "nc" not in _NC_CACHE:
        _NC_CACHE["nc"] = build_nc()
    return _NC_CACHE["nc"]


def kernel(**inputs):
    nc = _get_nc()
    in_maps = host_prep(inputs)
    res = run_bass_kernel_spmd(nc, in_maps, core_ids=list(range(NCORES)))
    out = np.empty((B, N, 3), np.float32)
    for b in range(B):
        for h in range(2):
            yT = res.results[2 * b + h]["yT"]
            out[b, h * NH:(h + 1) * NH] = yT[:, :NH].T
    return out

